# revision 23
# baseline (speedup 1.0000x reference)
"""Trainium2 Bass kernel for nn_MHA: 16-head MHA, B=4, S=2048, IN=1024, D=64.

Sharding: q-row data parallel across 8 cores. Core i handles batch b=i//2,
query rows [half*1024, half*1024+1024) with half=i%2. Each core computes its
disjoint slice of BOTH outputs (out rows and mean-probs rows), so no
collectives and no host-side reduction are needed -- only concatenation.

Input trick: the host sends each core xT = x[b].T with the kpos axis ROTATED
so that this core's query rows always occupy columns 0:1024. Attention is
invariant to a kpos permutation (softmax + sum over kpos); only the
mean-probs output columns come back permuted, which the host un-rotates.
This makes the program identical across cores (pure SPMD, no partition id).

Per-core pipeline, head-pair-outer so projections interleave with attention
(ACT exp work starts ~15us in, instead of after a ~140us projection phase):
  for each head pair hp (feature chunk hp of 128 = two heads):
    qT[hp] [feat,q]  = Wq[:,hp].T @ xT[:, :1024]   (PE, bf16)
    kT[hp] [feat,kp] = Wk[:,hp].T @ xT              (PE, bf16)
    (v projection spread over hp=0..3, 256 feature cols each)
    per 128-row q tile: S = qT.T @ kT  (row-group-packed K=64 pairs)
      exp + row-sum in one ACT pass (accum_out), fp16
      probs = exp * (1/(16*sum))      (DVE tensor_scalar, in place)
      out2 += probs                   (DVE fp16 accumulator)
      probsT via ONE DMA-xbar transpose per (hp, qtile) [both heads]
    attnT[hp] = v.T @ probsT          (PE fp16, PSUM-accumulated over kpos)
  out = attnT.T @ (16*Wo) + (host: bv @ Wo + bo)   (PE fp16)
  out2 -> f32 via SWDGE cast DMA
The 1/16 in the probs scaling cancels against the host-prescaled 16*Wo.
Host fix `out += bv @ Wo + bo` is valid because softmax rows sum to 1.
"""

import sys

if "/opt/trn_rl_repo" not in sys.path:
    sys.path.insert(0, "/opt/trn_rl_repo")

import numpy as np
import ml_dtypes

import concourse.bass as bass
import concourse.mybir as mybir
import concourse.tile as tile
from concourse import bacc
from concourse.bass_utils import run_bass_kernel_spmd

# Problem dims (hardcoded per contract)
B, S, IN, H, D = 4, 2048, 1024, 16, 64
NCORES = 8
QL = S // 2          # 1024 query rows per core
P = 128              # partitions
NIC = IN // P        # 8 input-feature chunks
NHP = H // 2         # 8 head pairs (one 128-wide feature chunk each)
NKB = S // P         # 16 kpos blocks
SM_SCALE = 1.0 / np.sqrt(np.float32(D))  # 0.125

BF16 = mybir.dt.bfloat16
FP16 = mybir.dt.float16
FP32 = mybir.dt.float32

# Module-level knobs (test.py may flip these before calling kernel()).
TRACE = False
LAST_RESULTS = None
# Timing-ablation flags (dev only; breaks numerics): set of strings among
# {"notrans", "nott", "nopv", "nos", "noexp"}.
ABLATE = frozenset()


def _emit(tc, t):
    """Emit the per-core MHA program. t: dict of dram APs."""
    nc = tc.nc
    from contextlib import ExitStack

    ctx = ExitStack()
    with ctx:
        # ---- pools live through the whole kernel --------------------------
        pers = ctx.enter_context(tc.tile_pool(name="pers", bufs=1))
        v_s = pers.tile([P, NKB, IN], FP16, tag="v")          # 32KB/part
        attnT_s = pers.tile([P, NHP, QL], FP16, tag="attnT")  # 16KB/part
        bq_t = pers.tile([P, NIC], FP32, tag="bq")
        bk_t = pers.tile([P, NIC], FP32, tag="bk")
        o2pool = ctx.enter_context(tc.tile_pool(name="o2", bufs=8))
        out2_acc = [
            o2pool.tile([P, S], FP16, tag="o2", name=f"o2_{i}") for i in range(8)
        ]

        nc.sync.dma_start(out=bq_t[:], in_=t["bq2"])
        nc.sync.dma_start(out=bk_t[:], in_=t["bk2"])

        with ExitStack() as hctx:
            xp = hctx.enter_context(tc.tile_pool(name="xp", bufs=1))
            qk = hctx.enter_context(tc.tile_pool(name="qk", bufs=2))
            wsl = hctx.enter_context(tc.tile_pool(name="wsl", bufs=2))
            work = hctx.enter_context(tc.tile_pool(name="work", bufs=2))
            ptp = hctx.enter_context(tc.tile_pool(name="ptp", bufs=1))
            small = hctx.enter_context(tc.tile_pool(name="small", bufs=8))
            pps = hctx.enter_context(tc.tile_pool(name="proj_ps", bufs=2, space="PSUM"))
            sps = hctx.enter_context(tc.tile_pool(name="s_ps", bufs=2, space="PSUM"))
            pvps = hctx.enter_context(tc.tile_pool(name="pv_ps", bufs=1, space="PSUM"))

            xT_s = xp.tile([P, NIC, S], BF16, tag="xT")       # 32KB/part
            # chunked load so the first projection matmul starts ~2us in
            xT_src = t["xT"].rearrange("(c p) k -> p c k", p=P)
            for c in range(NIC):
                nc.sync.dma_start(out=xT_s[:, c, :], in_=xT_src[:, c, :])

            for hp in range(NHP):
                # -- projections for this head pair --
                wq_sl = wsl.tile([P, NIC, P], BF16, tag="wq", name=f"wq_{hp}")
                wk_sl = wsl.tile([P, NIC, P], BF16, tag="wk", name=f"wk_{hp}")
                nc.sync.dma_start(
                    out=wq_sl[:],
                    in_=t["wq"].rearrange("(c p) f -> p c f", p=P)[
                        :, :, hp * P : (hp + 1) * P
                    ],
                )
                nc.sync.dma_start(
                    out=wk_sl[:],
                    in_=t["wk"].rearrange("(c p) f -> p c f", p=P)[
                        :, :, hp * P : (hp + 1) * P
                    ],
                )
                qTs = qk.tile([P, QL], BF16, tag="qT", name=f"qT_{hp}")
                kTs = qk.tile([P, S], BF16, tag="kT", name=f"kT_{hp}")
                for qc in range(QL // 512):
                    ps = pps.tile([P, 512], FP32, tag="ps", name=f"qps_{hp}_{qc}")
                    for c in range(NIC):
                        nc.tensor.matmul(
                            ps[:],
                            wq_sl[:, c, :],
                            xT_s[:, c, qc * 512 : (qc + 1) * 512],
                            start=(c == 0),
                            stop=(c == NIC - 1),
                        )
                    nc.vector.tensor_scalar_add(
                        qTs[:, qc * 512 : (qc + 1) * 512], ps[:], bq_t[:, hp : hp + 1]
                    )
                for kc in range(S // 512):
                    ps = pps.tile([P, 512], FP32, tag="ps", name=f"kps_{hp}_{kc}")
                    for c in range(NIC):
                        nc.tensor.matmul(
                            ps[:],
                            wk_sl[:, c, :],
                            xT_s[:, c, kc * 512 : (kc + 1) * 512],
                            start=(c == 0),
                            stop=(c == NIC - 1),
                        )
                    nc.vector.tensor_scalar_add(
                        kTs[:, kc * 512 : (kc + 1) * 512], ps[:], bk_t[:, hp : hp + 1]
                    )
                # -- v projection, spread over hp=0..3 (256 cols each) --
                if hp < 4:
                    wv_sl = wsl.tile([P, NIC, 256], BF16, tag="wv", name=f"wv_{hp}")
                    nc.sync.dma_start(
                        out=wv_sl[:],
                        in_=t["wv"].rearrange("(c p) f -> p c f", p=P)[
                            :, :, hp * 256 : (hp + 1) * 256
                        ],
                    )
                    for kb in range(NKB):
                        vps = pps.tile([P, 256], FP32, tag="ps", name=f"vps_{hp}_{kb}")
                        for c in range(NIC):
                            nc.tensor.matmul(
                                vps[:],
                                xT_s[:, c, kb * P : (kb + 1) * P],
                                wv_sl[:, c, :],
                                start=(c == 0),
                                stop=(c == NIC - 1),
                            )
                        nc.scalar.copy(
                            v_s[:, kb, hp * 256 : (hp + 1) * 256], vps[:]
                        )

                # -- attention for this head pair --
                for qb in range(2):
                    pT = ptp.tile(
                        [P, 2 * NKB, 512], FP16, tag="pT", name=f"pT_{hp}_{qb}"
                    )
                    for qt in range(4):
                        qg = qb * 4 + qt
                        ex = work.tile([P, 2, S], FP16, tag="ex", name=f"ex_{hp}_{qg}")
                        sums = [
                            small.tile([P, 2], FP32, tag=f"sm{h2}", name=f"sm_{hp}_{qg}_{h2}")
                            for h2 in range(2)
                        ]
                        for c in range(2):
                            s_ps = [
                                sps.tile([P, 1024], FP32, tag="sps", name=f"sps_{hp}_{qg}_{c}_{h2}")
                                for h2 in range(2)
                            ]
                            if "nos" not in ABLATE:
                                for cc in range(2):
                                    ks = slice(
                                        c * 1024 + cc * 512, c * 1024 + (cc + 1) * 512
                                    )
                                    for h2 in range(2):
                                        pr = h2 * 64
                                        nc.tensor.matmul(
                                            s_ps[h2][:, cc * 512 : (cc + 1) * 512],
                                            qTs[pr : pr + 64, qg * P : (qg + 1) * P],
                                            kTs[pr : pr + 64, ks],
                                            start=True,
                                            stop=True,
                                            tile_position=(pr, 0),
                                        )
                            if "noexp" not in ABLATE:
                                for h2 in range(2):
                                    nc.scalar.activation(
                                        ex[:, h2, c * 1024 : (c + 1) * 1024],
                                        s_ps[h2][:],
                                        mybir.ActivationFunctionType.Exp,
                                        scale=float(SM_SCALE),
                                        accum_out=sums[h2][:, c : c + 1],
                                    )
                        for h2 in range(2):
                            sum1 = small.tile([P, 1], FP32, tag="s1", name=f"s1_{hp}_{qg}_{h2}")
                            r16 = small.tile([P, 1], FP32, tag="r16", name=f"r16_{hp}_{qg}_{h2}")
                            nc.vector.reduce_sum(
                                sum1[:], sums[h2][:], axis=mybir.AxisListType.X
                            )
                            nc.vector.tensor_scalar_mul(sum1[:], sum1[:], 16.0)
                            nc.vector.reciprocal(r16[:], sum1[:])
                            nc.vector.tensor_scalar_mul(
                                ex[:, h2, :], ex[:, h2, :], r16[:, 0:1]
                            )
                        # mean-probs accumulation (fp16; cast to f32 on DMA out)
                        if "nott" not in ABLATE:
                            if hp == 0:
                                nc.vector.tensor_copy(out2_acc[qg][:], ex[:, 0, :])
                            else:
                                nc.vector.tensor_add(
                                    out2_acc[qg][:], out2_acc[qg][:], ex[:, 0, :]
                                )
                            nc.vector.tensor_add(
                                out2_acc[qg][:], out2_acc[qg][:], ex[:, 1, :]
                            )
                        elif hp == 0:
                            nc.vector.tensor_copy(out2_acc[qg][:], ex[:, 0, :])
                        # one transpose for both heads: [128, 4096] -> 32 blocks
                        if "notrans" not in ABLATE:
                            nc.sync.dma_start_transpose(
                                out=pT[:, :, qt * P : (qt + 1) * P], in_=ex[:]
                            )
                        elif qt == 0:
                            nc.vector.tensor_copy(pT[:, 0, 0:2], ex[:, 0, 0:2])
                    # PV for this (hp, qb): attnT[d, q] over kpos blocks.
                    # Separate PSUM banks per head (an accumulation group's
                    # start clears has_written for its whole bank).
                    pv = [
                        pvps.tile([P, 512], FP32, tag=f"pv{h2}", name=f"pv_{hp}_{qb}_{h2}")
                        for h2 in range(2)
                    ]
                    for kc in range(NKB if "nopv" not in ABLATE else 1):
                        for h2 in range(2):
                            h = hp * 2 + h2
                            pr = h2 * 64
                            nc.tensor.matmul(
                                pv[h2][pr : pr + 64, :],
                                v_s[:, kc, h * 64 : (h + 1) * 64],
                                pT[:, h2 * NKB + kc, :],
                                start=(kc == 0),
                                stop=(kc == NKB - 1 or "nopv" in ABLATE),
                                tile_position=(0, pr),
                            )
                    for h2 in range(2):
                        pr = h2 * 64
                        nc.scalar.copy(
                            attnT_s[pr : pr + 64, hp, qb * 512 : (qb + 1) * 512],
                            pv[h2][pr : pr + 64, :],
                        )

        # ---- output projection (xT/qk/pT pools freed above) ---------------
        with ExitStack() as octx:
            op = octx.enter_context(tc.tile_pool(name="op", bufs=2))
            wop = octx.enter_context(tc.tile_pool(name="wop", bufs=1))
            opps = octx.enter_context(tc.tile_pool(name="op_ps", bufs=2, space="PSUM"))
            wo_s = wop.tile([P, NHP, IN], FP16, tag="wo")
            nc.sync.dma_start(
                out=wo_s[:], in_=t["wo"].rearrange("(c p) f -> p c f", p=P)
            )
            for qg in range(8):
                ostage = op.tile([P, IN], FP32, tag="ostage", name=f"ost_{qg}")
                for oc in range(IN // 512):
                    ops = opps.tile([P, 512], FP32, tag="ops", name=f"ops_{qg}_{oc}")
                    for hp in range(NHP):
                        nc.tensor.matmul(
                            ops[:],
                            attnT_s[:, hp, qg * P : (qg + 1) * P],
                            wo_s[:, hp, oc * 512 : (oc + 1) * 512],
                            start=(hp == 0),
                            stop=(hp == NHP - 1),
                        )
                    nc.scalar.copy(ostage[:, oc * 512 : (oc + 1) * 512], ops[:])
                nc.sync.dma_start(
                    out=t["out_sl"][qg * P : (qg + 1) * P, :], in_=ostage[:]
                )
                # SWDGE DMA casts the fp16 accumulator to the f32 output
                nc.gpsimd.dma_start(
                    out=t["out2_sl"][qg * P : (qg + 1) * P, :], in_=out2_acc[qg][:]
                )


_BUILT = {}


def _build(repeats=1):
    key = (repeats, tuple(sorted(ABLATE)))
    if key in _BUILT:
        return _BUILT[key]
    # Bacc (not raw Bass): its compile() pass splits multi-semaphore waits
    # into EventSemaphore pre-waits -- walrus codegen allows only 1 wait per
    # compute instruction.
    nc = bacc.Bacc("TRN2", target_bir_lowering=False, debug=False)
    t = {}
    t["xT"] = nc.dram_tensor("xT", [IN, S], BF16, kind="ExternalInput").ap()
    t["wq"] = nc.dram_tensor("wq", [IN, IN], BF16, kind="ExternalInput").ap()
    t["wk"] = nc.dram_tensor("wk", [IN, IN], BF16, kind="ExternalInput").ap()
    t["wv"] = nc.dram_tensor("wv", [IN, IN], BF16, kind="ExternalInput").ap()
    t["wo"] = nc.dram_tensor("wo", [IN, IN], FP16, kind="ExternalInput").ap()
    t["bq2"] = nc.dram_tensor("bq2", [P, NIC], FP32, kind="ExternalInput").ap()
    t["bk2"] = nc.dram_tensor("bk2", [P, NIC], FP32, kind="ExternalInput").ap()
    t["out_sl"] = nc.dram_tensor("out_sl", [QL, IN], FP32, kind="ExternalOutput").ap()
    t["out2_sl"] = nc.dram_tensor("out2_sl", [QL, S], FP32, kind="ExternalOutput").ap()

    with tile.TileContext(nc) as tc:
        for _ in range(repeats):  # repeats>1: timing builds only
            _emit(tc, t)
    nc.compile()
    _BUILT[key] = nc
    return nc


def _host_prep(x, Wq, Wk, Wv, Wo, bq, bk):
    """Build the 8 per-core input maps."""
    bf = ml_dtypes.bfloat16
    wq_b = Wq.astype(bf)
    wk_b = Wk.astype(bf)
    wv_b = Wv.astype(bf)
    wo_h = (Wo * np.float32(16.0)).astype(np.float16)
    bq2 = np.ascontiguousarray(bq.reshape(NIC, P).T.astype(np.float32))
    bk2 = np.ascontiguousarray(bk.reshape(NIC, P).T.astype(np.float32))
    in_maps = []
    for b in range(B):
        xT_b = x[b].T.astype(bf)  # [IN, S]
        # rotate kpos so this core's query rows are columns 0:QL
        xT_rot = np.concatenate([xT_b[:, QL:], xT_b[:, :QL]], axis=1)
        for half in range(2):
            in_maps.append(
                {
                    "xT": xT_b if half == 0 else xT_rot,
                    "wq": wq_b,
                    "wk": wk_b,
                    "wv": wv_b,
                    "wo": wo_h,
                    "bq2": bq2,
                    "bk2": bk2,
                }
            )
    return in_maps


def kernel(x, Wq, bq, Wk, bk, Wv, bv, Wo, bo):
    global LAST_RESULTS
    x = np.asarray(x, dtype=np.float32)
    Wq = np.asarray(Wq, dtype=np.float32)
    Wk = np.asarray(Wk, dtype=np.float32)
    Wv = np.asarray(Wv, dtype=np.float32)
    Wo = np.asarray(Wo, dtype=np.float32)
    bq = np.asarray(bq, dtype=np.float32)
    bk = np.asarray(bk, dtype=np.float32)
    bv = np.asarray(bv, dtype=np.float32)
    bo = np.asarray(bo, dtype=np.float32)

    nc = _build()
    in_maps = _host_prep(x, Wq, Wk, Wv, Wo, bq, bk)
    # trace/NTFF profiling is unavailable in this container (no axon.trn
    # hook); run_bass_kernel_spmd with trace=False goes straight to PJRT.
    res = run_bass_kernel_spmd(
        nc, in_maps, core_ids=list(range(NCORES)), trace=False
    )
    LAST_RESULTS = res

    out = np.empty((B, S, IN), dtype=np.float32)
    out2 = np.empty((B, S, S), dtype=np.float32)
    for i in range(NCORES):
        b, half = i // 2, i % 2
        rows = slice(half * QL, (half + 1) * QL)
        out[b, rows, :] = res.results[i]["out_sl"]
        o2 = res.results[i]["out2_sl"]
        if half == 1:
            # un-rotate the kpos axis (device saw kpos rotated left by QL)
            o2 = np.concatenate([o2[:, QL:], o2[:, :QL]], axis=1)
        out2[b, rows, :] = o2

    # host bias fixes: probs rows sum to 1 -> attn bias = bv @ Wo; plus bo.
    out += (bv @ Wo + bo)[None, None, :]
    return out, out2


# revision 24
# speedup vs baseline: 1.1598x; 1.1598x over previous
"""Trainium2 Bass kernel for nn_MHA: 16-head MHA, B=4, S=2048, IN=1024, D=64.

Sharding: q-row data parallel across 8 cores. Core i handles batch b=i//2,
query rows [half*1024, half*1024+1024) with half=i%2. Each core computes its
disjoint slice of BOTH outputs (out rows and mean-probs rows), so no
collectives and no host-side reduction are needed -- only concatenation.

Input trick: the host sends each core xT = x[b].T with the kpos axis ROTATED
so that this core's query rows always occupy columns 0:1024. Attention is
invariant to a kpos permutation (softmax + sum over kpos); only the
mean-probs output columns come back permuted, which the host un-rotates.
This makes the program identical across cores (pure SPMD, no partition id).

Per-core pipeline, head-pair-outer so projections interleave with attention
(ACT exp work starts ~15us in, instead of after a ~140us projection phase):
  for each head pair hp (feature chunk hp of 128 = two heads):
    qT[hp] [feat,q]  = Wq[:,hp].T @ xT[:, :1024]   (PE, bf16)
    kT[hp] [feat,kp] = Wk[:,hp].T @ xT              (PE, bf16)
    (v projection spread over hp=0..3, 256 feature cols each)
    per 128-row q tile: S = qT.T @ kT  (row-group-packed K=64 pairs)
      exp + row-sum in one ACT pass (accum_out), fp16
      probs = exp * (1/(16*sum))      (DVE tensor_scalar, in place)
      out2 += probs                   (DVE fp16 accumulator)
      probsT via ONE DMA-xbar transpose per (hp, qtile) [both heads]
    attnT[hp] = v.T @ probsT          (PE fp16, PSUM-accumulated over kpos)
  out = attnT.T @ (16*Wo) + (host: bv @ Wo + bo)   (PE fp16)
  out2 -> f32 via SWDGE cast DMA
The 1/16 in the probs scaling cancels against the host-prescaled 16*Wo.
Host fix `out += bv @ Wo + bo` is valid because softmax rows sum to 1.
"""

import sys

if "/opt/trn_rl_repo" not in sys.path:
    sys.path.insert(0, "/opt/trn_rl_repo")

import numpy as np
import ml_dtypes

import concourse.bass as bass
import concourse.mybir as mybir
import concourse.tile as tile
from concourse import bacc
from concourse.bass_utils import run_bass_kernel_spmd

# Problem dims (hardcoded per contract)
B, S, IN, H, D = 4, 2048, 1024, 16, 64
NCORES = 8
QL = S // 2          # 1024 query rows per core
P = 128              # partitions
NIC = IN // P        # 8 input-feature chunks
NHP = H // 2         # 8 head pairs (one 128-wide feature chunk each)
NKB = S // P         # 16 kpos blocks
SM_SCALE = 1.0 / np.sqrt(np.float32(D))  # 0.125

BF16 = mybir.dt.bfloat16
FP16 = mybir.dt.float16
FP32 = mybir.dt.float32

# Module-level knobs (test.py may flip these before calling kernel()).
TRACE = False
LAST_RESULTS = None
# Timing-ablation flags (dev only; breaks numerics): set of strings among
# {"notrans", "nott", "nopv", "nos", "noexp"}.
ABLATE = frozenset()


def _emit(tc, t):
    """Emit the per-core MHA program. t: dict of dram APs."""
    nc = tc.nc
    from contextlib import ExitStack

    ctx = ExitStack()
    with ctx:
        # ---- pools live through the whole kernel --------------------------
        pers = ctx.enter_context(tc.tile_pool(name="pers", bufs=1))
        v_s = pers.tile([P, NKB, IN], FP16, tag="v")          # 32KB/part
        attnT_s = pers.tile([P, NHP, QL], FP16, tag="attnT")  # 16KB/part
        bq_t = pers.tile([P, NIC], FP32, tag="bq")
        bk_t = pers.tile([P, NIC], FP32, tag="bk")
        o2pool = ctx.enter_context(tc.tile_pool(name="o2", bufs=8))
        out2_acc = [
            o2pool.tile([P, S], FP16, tag="o2", name=f"o2_{i}") for i in range(8)
        ]

        nc.sync.dma_start(out=bq_t[:], in_=t["bq2"])
        nc.sync.dma_start(out=bk_t[:], in_=t["bk2"])

        with ExitStack() as hctx:
            xp = hctx.enter_context(tc.tile_pool(name="xp", bufs=1))
            qk = hctx.enter_context(tc.tile_pool(name="qk", bufs=2))
            wsl = hctx.enter_context(tc.tile_pool(name="wsl", bufs=2))
            work = hctx.enter_context(tc.tile_pool(name="work", bufs=3))
            ptp = hctx.enter_context(tc.tile_pool(name="ptp", bufs=1))
            small = hctx.enter_context(tc.tile_pool(name="small", bufs=8))
            pps = hctx.enter_context(tc.tile_pool(name="proj_ps", bufs=2, space="PSUM"))
            sps = hctx.enter_context(tc.tile_pool(name="s_ps", bufs=2, space="PSUM"))
            pvps = hctx.enter_context(tc.tile_pool(name="pv_ps", bufs=1, space="PSUM"))

            xT_s = xp.tile([P, NIC, S], BF16, tag="xT")       # 32KB/part
            # chunked load so the first projection matmul starts ~2us in
            xT_src = t["xT"].rearrange("(c p) k -> p c k", p=P)

            for hp in range(NHP):
                # -- projections for this head pair --
                wq_sl = wsl.tile([P, NIC, P], BF16, tag="wq", name=f"wq_{hp}")
                wk_sl = wsl.tile([P, NIC, P], BF16, tag="wk", name=f"wk_{hp}")
                nc.sync.dma_start(
                    out=wq_sl[:],
                    in_=t["wq"].rearrange("(c p) f -> p c f", p=P)[
                        :, :, hp * P : (hp + 1) * P
                    ],
                )
                nc.sync.dma_start(
                    out=wk_sl[:],
                    in_=t["wk"].rearrange("(c p) f -> p c f", p=P)[
                        :, :, hp * P : (hp + 1) * P
                    ],
                )
                if hp == 0:
                    for c in range(NIC):
                        nc.sync.dma_start(out=xT_s[:, c, :], in_=xT_src[:, c, :])
                qTs = qk.tile([P, QL], BF16, tag="qT", name=f"qT_{hp}")
                kTs = qk.tile([P, S], BF16, tag="kT", name=f"kT_{hp}")
                for qc in range(QL // 512):
                    ps = pps.tile([P, 512], FP32, tag="ps", name=f"qps_{hp}_{qc}")
                    for c in range(NIC):
                        nc.tensor.matmul(
                            ps[:],
                            wq_sl[:, c, :],
                            xT_s[:, c, qc * 512 : (qc + 1) * 512],
                            start=(c == 0),
                            stop=(c == NIC - 1),
                        )
                    nc.vector.tensor_scalar_add(
                        qTs[:, qc * 512 : (qc + 1) * 512], ps[:], bq_t[:, hp : hp + 1]
                    )
                for kc in range(S // 512):
                    ps = pps.tile([P, 512], FP32, tag="ps", name=f"kps_{hp}_{kc}")
                    for c in range(NIC):
                        nc.tensor.matmul(
                            ps[:],
                            wk_sl[:, c, :],
                            xT_s[:, c, kc * 512 : (kc + 1) * 512],
                            start=(c == 0),
                            stop=(c == NIC - 1),
                        )
                    nc.vector.tensor_scalar_add(
                        kTs[:, kc * 512 : (kc + 1) * 512], ps[:], bk_t[:, hp : hp + 1]
                    )
                # -- v projection, spread over hp=0..3 (256 cols each) --
                if hp < 4:
                    wv_sl = wsl.tile([P, NIC, 256], BF16, tag="wv", name=f"wv_{hp}")
                    nc.sync.dma_start(
                        out=wv_sl[:],
                        in_=t["wv"].rearrange("(c p) f -> p c f", p=P)[
                            :, :, hp * 256 : (hp + 1) * 256
                        ],
                    )
                    for kb in range(NKB):
                        vps = pps.tile([P, 256], FP32, tag="ps", name=f"vps_{hp}_{kb}")
                        for c in range(NIC):
                            nc.tensor.matmul(
                                vps[:],
                                xT_s[:, c, kb * P : (kb + 1) * P],
                                wv_sl[:, c, :],
                                start=(c == 0),
                                stop=(c == NIC - 1),
                            )
                        nc.scalar.copy(
                            v_s[:, kb, hp * 256 : (hp + 1) * 256], vps[:]
                        )

                # -- attention for this head pair --
                for qb in range(2):
                    pT = ptp.tile(
                        [P, 2 * NKB, 512], FP16, tag="pT", name=f"pT_{hp}_{qb}"
                    )
                    for qt in range(4):
                        qg = qb * 4 + qt
                        ex = work.tile([P, 2, S], FP16, tag="ex", name=f"ex_{hp}_{qg}")
                        sums = [
                            small.tile([P, 2], FP32, tag=f"sm{h2}", name=f"sm_{hp}_{qg}_{h2}")
                            for h2 in range(2)
                        ]
                        for c in range(2):
                            s_ps = [
                                sps.tile([P, 1024], FP32, tag="sps", name=f"sps_{hp}_{qg}_{c}_{h2}")
                                for h2 in range(2)
                            ]
                            if "nos" not in ABLATE:
                                for cc in range(2):
                                    ks = slice(
                                        c * 1024 + cc * 512, c * 1024 + (cc + 1) * 512
                                    )
                                    for h2 in range(2):
                                        pr = h2 * 64
                                        nc.tensor.matmul(
                                            s_ps[h2][:, cc * 512 : (cc + 1) * 512],
                                            qTs[pr : pr + 64, qg * P : (qg + 1) * P],
                                            kTs[pr : pr + 64, ks],
                                            start=True,
                                            stop=True,
                                            tile_position=(pr, 0),
                                        )
                            if "noexp" not in ABLATE:
                                for h2 in range(2):
                                    nc.scalar.activation(
                                        ex[:, h2, c * 1024 : (c + 1) * 1024],
                                        s_ps[h2][:],
                                        mybir.ActivationFunctionType.Exp,
                                        scale=float(SM_SCALE),
                                        accum_out=sums[h2][:, c : c + 1],
                                    )
                        for h2 in range(2):
                            sum1 = small.tile([P, 1], FP32, tag="s1", name=f"s1_{hp}_{qg}_{h2}")
                            r16 = small.tile([P, 1], FP32, tag="r16", name=f"r16_{hp}_{qg}_{h2}")
                            nc.vector.reduce_sum(
                                sum1[:], sums[h2][:], axis=mybir.AxisListType.X
                            )
                            nc.vector.tensor_scalar_mul(sum1[:], sum1[:], 16.0)
                            nc.vector.reciprocal(r16[:], sum1[:])
                            nc.vector.tensor_scalar_mul(
                                ex[:, h2, :], ex[:, h2, :], r16[:, 0:1]
                            )
                        # mean-probs accumulation (fp16; cast to f32 on DMA out)
                        if "nott" not in ABLATE:
                            if hp == 0:
                                nc.vector.tensor_copy(out2_acc[qg][:], ex[:, 0, :])
                            else:
                                nc.vector.tensor_add(
                                    out2_acc[qg][:], out2_acc[qg][:], ex[:, 0, :]
                                )
                            nc.vector.tensor_add(
                                out2_acc[qg][:], out2_acc[qg][:], ex[:, 1, :]
                            )
                        elif hp == 0:
                            nc.vector.tensor_copy(out2_acc[qg][:], ex[:, 0, :])
                        if hp == NHP - 1:
                            # SWDGE DMA casts fp16 acc to the f32 output
                            nc.gpsimd.dma_start(
                                out=t["out2_sl"][qg * P : (qg + 1) * P, :],
                                in_=out2_acc[qg][:],
                            )
                        # one transpose for both heads: [128, 4096] -> 32 blocks
                        if "notrans" not in ABLATE:
                            nc.sync.dma_start_transpose(
                                out=pT[:, :, qt * P : (qt + 1) * P], in_=ex[:]
                            )
                        elif qt == 0:
                            nc.vector.tensor_copy(pT[:, 0, 0:2], ex[:, 0, 0:2])
                    # PV for this (hp, qb): attnT[d, q] over kpos blocks.
                    # Separate PSUM banks per head (an accumulation group's
                    # start clears has_written for its whole bank).
                    pv = [
                        pvps.tile([P, 512], FP32, tag=f"pv{h2}", name=f"pv_{hp}_{qb}_{h2}")
                        for h2 in range(2)
                    ]
                    for kc in range(NKB if "nopv" not in ABLATE else 1):
                        for h2 in range(2):
                            h = hp * 2 + h2
                            pr = h2 * 64
                            nc.tensor.matmul(
                                pv[h2][pr : pr + 64, :],
                                v_s[:, kc, h * 64 : (h + 1) * 64],
                                pT[:, h2 * NKB + kc, :],
                                start=(kc == 0),
                                stop=(kc == NKB - 1 or "nopv" in ABLATE),
                                tile_position=(0, pr),
                            )
                    for h2 in range(2):
                        pr = h2 * 64
                        nc.scalar.copy(
                            attnT_s[pr : pr + 64, hp, qb * 512 : (qb + 1) * 512],
                            pv[h2][pr : pr + 64, :],
                        )

        # ---- output projection (xT/qk/pT pools freed above) ---------------
        with ExitStack() as octx:
            op = octx.enter_context(tc.tile_pool(name="op", bufs=2))
            wop = octx.enter_context(tc.tile_pool(name="wop", bufs=1))
            opps = octx.enter_context(tc.tile_pool(name="op_ps", bufs=2, space="PSUM"))
            wo_s = wop.tile([P, NHP, IN], FP16, tag="wo")
            nc.sync.dma_start(
                out=wo_s[:], in_=t["wo"].rearrange("(c p) f -> p c f", p=P)
            )
            for qg in range(8):
                ostage = op.tile([P, IN], FP32, tag="ostage", name=f"ost_{qg}")
                for oc in range(IN // 512):
                    ops = opps.tile([P, 512], FP32, tag="ops", name=f"ops_{qg}_{oc}")
                    for hp in range(NHP):
                        nc.tensor.matmul(
                            ops[:],
                            attnT_s[:, hp, qg * P : (qg + 1) * P],
                            wo_s[:, hp, oc * 512 : (oc + 1) * 512],
                            start=(hp == 0),
                            stop=(hp == NHP - 1),
                        )
                    nc.scalar.copy(ostage[:, oc * 512 : (oc + 1) * 512], ops[:])
                nc.sync.dma_start(
                    out=t["out_sl"][qg * P : (qg + 1) * P, :], in_=ostage[:]
                )


_BUILT = {}


def _build(repeats=1):
    key = (repeats, tuple(sorted(ABLATE)))
    if key in _BUILT:
        return _BUILT[key]
    # Bacc (not raw Bass): its compile() pass splits multi-semaphore waits
    # into EventSemaphore pre-waits -- walrus codegen allows only 1 wait per
    # compute instruction.
    nc = bacc.Bacc("TRN2", target_bir_lowering=False, debug=False)
    t = {}
    t["xT"] = nc.dram_tensor("xT", [IN, S], BF16, kind="ExternalInput").ap()
    t["wq"] = nc.dram_tensor("wq", [IN, IN], BF16, kind="ExternalInput").ap()
    t["wk"] = nc.dram_tensor("wk", [IN, IN], BF16, kind="ExternalInput").ap()
    t["wv"] = nc.dram_tensor("wv", [IN, IN], BF16, kind="ExternalInput").ap()
    t["wo"] = nc.dram_tensor("wo", [IN, IN], FP16, kind="ExternalInput").ap()
    t["bq2"] = nc.dram_tensor("bq2", [P, NIC], FP32, kind="ExternalInput").ap()
    t["bk2"] = nc.dram_tensor("bk2", [P, NIC], FP32, kind="ExternalInput").ap()
    t["out_sl"] = nc.dram_tensor("out_sl", [QL, IN], FP32, kind="ExternalOutput").ap()
    t["out2_sl"] = nc.dram_tensor("out2_sl", [QL, S], FP32, kind="ExternalOutput").ap()

    with tile.TileContext(nc) as tc:
        for _ in range(repeats):  # repeats>1: timing builds only
            _emit(tc, t)
    nc.compile()
    _BUILT[key] = nc
    return nc


def _host_prep(x, Wq, Wk, Wv, Wo, bq, bk):
    """Build the 8 per-core input maps."""
    bf = ml_dtypes.bfloat16
    wq_b = Wq.astype(bf)
    wk_b = Wk.astype(bf)
    wv_b = Wv.astype(bf)
    wo_h = (Wo * np.float32(16.0)).astype(np.float16)
    bq2 = np.ascontiguousarray(bq.reshape(NIC, P).T.astype(np.float32))
    bk2 = np.ascontiguousarray(bk.reshape(NIC, P).T.astype(np.float32))
    in_maps = []
    for b in range(B):
        xT_b = x[b].T.astype(bf)  # [IN, S]
        # rotate kpos so this core's query rows are columns 0:QL
        xT_rot = np.concatenate([xT_b[:, QL:], xT_b[:, :QL]], axis=1)
        for half in range(2):
            in_maps.append(
                {
                    "xT": xT_b if half == 0 else xT_rot,
                    "wq": wq_b,
                    "wk": wk_b,
                    "wv": wv_b,
                    "wo": wo_h,
                    "bq2": bq2,
                    "bk2": bk2,
                }
            )
    return in_maps


def kernel(x, Wq, bq, Wk, bk, Wv, bv, Wo, bo):
    global LAST_RESULTS
    x = np.asarray(x, dtype=np.float32)
    Wq = np.asarray(Wq, dtype=np.float32)
    Wk = np.asarray(Wk, dtype=np.float32)
    Wv = np.asarray(Wv, dtype=np.float32)
    Wo = np.asarray(Wo, dtype=np.float32)
    bq = np.asarray(bq, dtype=np.float32)
    bk = np.asarray(bk, dtype=np.float32)
    bv = np.asarray(bv, dtype=np.float32)
    bo = np.asarray(bo, dtype=np.float32)

    nc = _build()
    in_maps = _host_prep(x, Wq, Wk, Wv, Wo, bq, bk)
    # trace/NTFF profiling is unavailable in this container (no axon.trn
    # hook); run_bass_kernel_spmd with trace=False goes straight to PJRT.
    res = run_bass_kernel_spmd(
        nc, in_maps, core_ids=list(range(NCORES)), trace=False
    )
    LAST_RESULTS = res

    out = np.empty((B, S, IN), dtype=np.float32)
    out2 = np.empty((B, S, S), dtype=np.float32)
    for i in range(NCORES):
        b, half = i // 2, i % 2
        rows = slice(half * QL, (half + 1) * QL)
        out[b, rows, :] = res.results[i]["out_sl"]
        o2 = res.results[i]["out2_sl"]
        if half == 1:
            # un-rotate the kpos axis (device saw kpos rotated left by QL)
            o2 = np.concatenate([o2[:, QL:], o2[:, :QL]], axis=1)
        out2[b, rows, :] = o2

    # host bias fixes: probs rows sum to 1 -> attn bias = bv @ Wo; plus bo.
    out += (bv @ Wo + bo)[None, None, :]
    return out, out2
